# revision 12
# baseline (speedup 1.0000x reference)
"""Causal self-attention (B=2, T=2048, C=1024, H=16) on 8 TRN2 NeuronCores.

Megatron-style tensor parallelism over heads: each core computes 2 of the 16
heads (Wq/Wk/Wv column-sharded, Wo row-sharded) and produces a partial output
projection; the partials are summed on the host (the all-reduce).

Per-core device dataflow (everything kept transposed so the PE contraction dim
is always the partition dim, no on-device transposes of x needed):
  xT [C, B*T] (host-pretransposed, replicated)
  QT/KT/VT = W_locT.T @ xT  (fp32r matmuls, K-tiled over C)
  V tiles   = PE-transpose of VT, with a ones-column appended per head
  S^T       = K_loc @ Q_loc^T per (batch, head, 128-j-tile, 512-i-block),
              2 heads packed in PE row groups (contraction = D = 64)
  P^T       = exp(S^T / 8) on ACT (no max-subtraction needed: |S|<~2),
              causal mask applied to diagonal block-tiles via mask multiply
  O^T|s     = [V|1].T @ P^T accumulated over j (ones row gives softmax sums)
  ylocT     = O^T * (1/s)  (reciprocal + PE broadcast of 1/s over partitions)
  yT_part   = Wo_locT.T @ ylocT   -> DRAM [C, B*T]
Host: y = (sum_cores yT_part).T + bo, reshape to [B, T, C].
"""

import sys

if "/opt/trn_rl_repo" not in sys.path:
    sys.path.insert(0, "/opt/trn_rl_repo")

import numpy as np

import concourse.bass as bass
import concourse.tile as tile
from concourse import bacc
from concourse import mybir
from concourse.bass_utils import run_bass_kernel_spmd

F32 = mybir.dt.float32
F32R = mybir.dt.float32r
AF = mybir.ActivationFunctionType
ALU = mybir.AluOpType

B, T, C, H = 2, 2048, 1024, 16
D = C // H          # 64
NCORES = 8
HL = H // NCORES    # 2 local heads
CL = C // NCORES    # 128 local channels
BT = B * T          # 4096
TB = 512            # t-block (matmul moving width, fp32 max)
NTB = BT // TB      # 8
NKT = C // 128      # 8 contraction tiles for projections
IB = T // TB        # 4 i-blocks per batch
NJT = T // 128      # 16 j-tiles per batch
VW = 130            # V tile width: 2 heads x (64 V cols + 1 ones col)


def build_nc() -> bass.Bass:
    nc = bacc.Bacc()

    xT_d = nc.declare_dram_parameter("xT", [C, BT], F32R, isOutput=False)
    wqT_d = nc.declare_dram_parameter("wqT", [128, C], F32R, isOutput=False)
    wkT_d = nc.declare_dram_parameter("wkT", [128, C], F32R, isOutput=False)
    wvT_d = nc.declare_dram_parameter("wvT", [128, C], F32R, isOutput=False)
    woT_d = nc.declare_dram_parameter("woT", [CL, C], F32R, isOutput=False)
    bq_d = nc.declare_dram_parameter("bq", [CL, 1], F32, isOutput=False)
    bk_d = nc.declare_dram_parameter("bk", [CL, 1], F32, isOutput=False)
    bv_d = nc.declare_dram_parameter("bv", [CL, 1], F32, isOutput=False)
    mask_d = nc.declare_dram_parameter("masks", [128, 4 * 1024], F32R, isOutput=False)
    id_d = nc.declare_dram_parameter("ident", [128, 128], F32R, isOutput=False)
    yT_d = nc.declare_dram_parameter("yT", [C, BT], F32, isOutput=True)

    with tile.TileContext(nc) as tc:
        with (
            tc.tile_pool(name="const", bufs=1) as const,
            tc.tile_pool(name="work", bufs=2) as work,
            tc.tile_pool(name="psum", bufs=2, space="PSUM") as psum,
        ):
            # ---------------- constants / persistent state ----------------
            wq_sb = const.tile([128, C], F32R)
            wk_sb = const.tile([128, C], F32R)
            wv_sb = const.tile([128, C], F32R)
            nc.sync.dma_start(wq_sb[:, :], wqT_d[:, :])
            nc.sync.dma_start(wk_sb[:, :], wkT_d[:, :])
            nc.sync.dma_start(wv_sb[:, :], wvT_d[:, :])
            wo_sb = const.tile([128, C], F32R)
            nc.sync.dma_start(wo_sb[:, :], woT_d[:, :])
            mask_sb = const.tile([128, 4 * 1024], F32R)
            nc.sync.dma_start(mask_sb[:, :], mask_d[:, :])
            id_sb = const.tile([128, 128], F32R)
            nc.sync.dma_start(id_sb[:, :], id_d[:, :])
            bq_sb = const.tile([128, 1], F32)
            nc.sync.dma_start(bq_sb[:, :], bq_d[:, :])
            bk_sb = const.tile([128, 1], F32)
            nc.sync.dma_start(bk_sb[:, :], bk_d[:, :])
            bv_sb = const.tile([128, 1], F32)
            nc.sync.dma_start(bv_sb[:, :], bv_d[:, :])
            ones_sb = const.tile([1, 64], F32R)
            nc.vector.memset(ones_sb[:, :].bitcast(mybir.dt.uint32), 0x3F800000)

            QT = const.tile([128, BT], F32R)
            KT = const.tile([128, BT], F32R)
            ylocT = const.tile([128, BT], F32R)
            V = const.tile([128, (BT // 128) * VW], F32R)
            # 1.0f bit pattern; ones columns survive the V copies below
            nc.gpsimd.memset(V[:, :].bitcast(mybir.dt.uint32), 0x3F800000)

            # ---------------- phase 1: Q/K/V projections -------------------
            for tb in range(NTB):
                tcols = slice(tb * TB, (tb + 1) * TB)
                xt = work.tile([128, NKT * TB], F32R, tag="xt", bufs=2,
                               name=f"xt_{tb}")
                xv = xT_d[:, tcols].rearrange("(ct p) t -> p ct t", p=128)
                xo = xt[:, :].rearrange("p (ct t) -> p ct t", ct=NKT)
                nc.sync.dma_start(xo[:, 0:NKT // 2, :], xv[:, 0:NKT // 2, :])
                nc.sync.dma_start(xo[:, NKT // 2:, :], xv[:, NKT // 2:, :])
                for which, w_sb, b_sb in (
                    ("q", wq_sb, bq_sb), ("k", wk_sb, bk_sb), ("v", wv_sb, bv_sb)
                ):
                    ps = psum.tile([128, TB], F32, tag="mm", name=f"ps_{which}_{tb}")
                    for ct in range(NKT):
                        nc.tensor.matmul(
                            ps[:, :],
                            w_sb[:, ct * 128:(ct + 1) * 128],
                            xt[:, ct * TB:(ct + 1) * TB],
                            start=(ct == 0), stop=(ct == NKT - 1),
                        )
                    if which == "q":
                        nc.vector.tensor_scalar_add(QT[:, tcols], ps[:, :], b_sb[:, :])
                    elif which == "k":
                        nc.vector.tensor_scalar_add(KT[:, tcols], ps[:, :], b_sb[:, :])
                    else:
                        vt_sb = work.tile([128, TB], F32R, tag="vtsb",
                                          name=f"vt_{tb}")
                        nc.vector.tensor_scalar_add(vt_sb[:, :], ps[:, :], b_sb[:, :])
                        for q in range(4):
                            jg = tb * 4 + q
                            tp = psum.tile([128, 128], F32R, tag="mm",
                                           name=f"tp_{jg}")
                            nc.tensor.transpose(
                                tp[:, :],
                                vt_sb[:, q * 128:(q + 1) * 128],
                                id_sb[:, :],
                            )
                            off = jg * VW
                            nc.vector.tensor_copy(V[:, off:off + 64], tp[:, 0:64])
                            nc.vector.tensor_copy(V[:, off + 65:off + 129],
                                                  tp[:, 64:128])

            # ------- phase 2+3: attention + output projection per i-block ----
            for b in range(B):
                for ib in range(IB):
                    i0 = b * T + ib * TB
                    icols = slice(i0, i0 + TB)
                    njt = 4 * (ib + 1)
                    ots = [
                        psum.tile([65, TB], F32, tag="ot", name=f"ot_{b}_{ib}_{h}")
                        for h in range(HL)
                    ]
                    for jt in range(njt):
                        jg = b * NJT + jt
                        st = psum.tile([128, 2 * TB], F32, tag="st",
                                       name=f"st_{b}_{ib}_{jt}")
                        for h in range(HL):
                            hs = slice(h * D, (h + 1) * D)
                            nc.tensor.matmul(
                                st[:, h * TB:(h + 1) * TB],
                                KT[hs, jg * 128:(jg + 1) * 128],
                                QT[hs, icols],
                                start=True, stop=True,
                            )
                        pt = work.tile([128, 2 * TB], F32R, tag="pt", bufs=4,
                                       name=f"pt_{b}_{ib}_{jt}")
                        if jt >= njt - 4:
                            q = jt - (njt - 4)
                            et = work.tile([128, 2 * TB], F32R, tag="et",
                                           name=f"et_{b}_{ib}_{jt}")
                            nc.scalar.activation(et[:, :], st[:, :], AF.Exp,
                                                 scale=0.125)
                            nc.vector.tensor_mul(
                                pt[:, :], et[:, :],
                                mask_sb[:, q * 1024:(q + 1) * 1024])
                        else:
                            nc.scalar.activation(pt[:, :], st[:, :], AF.Exp,
                                                 scale=0.125)
                        for h in range(HL):
                            off = jg * VW + h * 65
                            nc.tensor.matmul(
                                ots[h][:, :],
                                V[:, off:off + 65],
                                pt[:, h * TB:(h + 1) * TB],
                                start=(jt == 0), stop=(jt == njt - 1),
                            )
                    for h in range(HL):
                        r_sb = work.tile([1, TB], F32R, tag="r", name=f"r_{b}_{ib}_{h}")
                        with nc.allow_low_precision(reason="fp32r 1/s for bcast"):
                            nc.vector.reciprocal(r_sb[:, :], ots[h][64:65, :])
                        bc = psum.tile([64, TB], F32, tag="mm",
                                       name=f"bc_{b}_{ib}_{h}")
                        nc.tensor.matmul(bc[:, :], ones_sb[:, :], r_sb[:, :],
                                         start=True, stop=True)
                        bc_sb = work.tile([64, TB], F32, tag="bcsb",
                                          name=f"bcs_{b}_{ib}_{h}")
                        nc.vector.tensor_copy(bc_sb[:, :], bc[:, :])
                        bc = bc_sb
                        if h == 0:
                            nc.vector.tensor_tensor(
                                ylocT[0:64, icols], ots[h][0:64, :], bc[:, :],
                                ALU.mult)
                        else:
                            yn1 = work.tile([64, TB], F32R, tag="yn1",
                                            name=f"yn_{b}_{ib}")
                            nc.vector.tensor_tensor(
                                yn1[:, :], ots[h][0:64, :], bc[:, :], ALU.mult)
                            nc.sync.dma_start(ylocT[64:128, icols], yn1[:, :])
                    # output projection for this i-block's 512 tokens
                    for co in range(8):
                        yp = psum.tile([128, TB], F32, tag="mm",
                                       name=f"yp_{b}_{ib}_{co}")
                        nc.tensor.matmul(
                            yp[:, :],
                            wo_sb[:, co * 128:(co + 1) * 128],
                            ylocT[:, icols],
                            start=True, stop=True,
                        )
                        yo = work.tile([128, TB], F32, tag="yo", bufs=3,
                                       name=f"yo_{b}_{ib}_{co}")
                        nc.vector.tensor_copy(yo[:, :], yp[:, :])
                        nc.sync.dma_start(yT_d[co * 128:(co + 1) * 128, icols],
                                          yo[:, :])
    nc.compile()
    return nc


def _host_inputs(x, Wq, bq, Wk, bk, Wv, bv, Wo):
    """Build the 8 per-core input maps (host-side layout prep + sharding)."""
    xT = np.ascontiguousarray(x.reshape(BT, C).T.astype(np.float32))
    masks = np.zeros((128, 4 * 1024), np.float32)
    jj = np.arange(128, dtype=np.int32)[:, None]
    ii = np.arange(TB, dtype=np.int32)[None, :]
    for q in range(4):
        m = (ii >= 128 * q + jj).astype(np.float32)
        masks[:, q * 1024:q * 1024 + TB] = m
        masks[:, q * 1024 + TB:(q + 1) * 1024] = m
    ident = np.eye(128, dtype=np.float32)

    def wtile(W, rows):
        # device layout: w_sb[p, k*128 + j] = W[rows][j, k*128 + p]
        wT = W[rows, :].T.astype(np.float32)          # [C, CL]
        return np.ascontiguousarray(
            wT.reshape(NKT, 128, CL).transpose(1, 0, 2).reshape(128, NKT * CL))

    in_maps = []
    for core in range(NCORES):
        rows = slice(core * CL, (core + 1) * CL)
        in_maps.append({
            "xT": xT,
            "wqT": wtile(Wq, rows),
            "wkT": wtile(Wk, rows),
            "wvT": wtile(Wv, rows),
            "woT": np.ascontiguousarray(Wo[:, rows].T.astype(np.float32)),
            "bq": np.ascontiguousarray(bq[rows].reshape(CL, 1).astype(np.float32)),
            "bk": np.ascontiguousarray(bk[rows].reshape(CL, 1).astype(np.float32)),
            "bv": np.ascontiguousarray(bv[rows].reshape(CL, 1).astype(np.float32)),
            "masks": masks,
            "ident": ident,
        })
    return in_maps


_NC_CACHE = None


def _get_nc():
    global _NC_CACHE
    if _NC_CACHE is None:
        _NC_CACHE = build_nc()
    return _NC_CACHE


def _run(inputs, trace=False):
    x = np.asarray(inputs["x"], np.float32)
    in_maps = _host_inputs(
        x,
        np.asarray(inputs["Wq"], np.float32), np.asarray(inputs["bq"], np.float32),
        np.asarray(inputs["Wk"], np.float32), np.asarray(inputs["bk"], np.float32),
        np.asarray(inputs["Wv"], np.float32), np.asarray(inputs["bv"], np.float32),
        np.asarray(inputs["Wo"], np.float32),
    )
    res = run_bass_kernel_spmd(_get_nc(), in_maps, list(range(NCORES)), trace=trace)
    yT = np.zeros((C, BT), np.float64)
    for core in range(NCORES):
        yT += res.results[core]["yT"].astype(np.float64)
    y = yT.T.astype(np.float32) + np.asarray(inputs["bo"], np.float32)
    return y.reshape(B, T, C), res


def kernel(**inputs) -> np.ndarray:
    out, _ = _run(inputs, trace=False)
    return out


def _install_profile_hook():
    """Register the axon NTFF profile hook (the agent image ships the ctypes
    shim in trn_agent_boot but lacks the antenv.axon_hooks module)."""
    import types

    if "antenv.axon_hooks" in sys.modules:
        return
    sys.path.insert(0, "/root/.axon_site")
    from trn_agent_boot.trn_boot import _ntff_profile_via_ctypes

    mod = types.ModuleType("antenv.axon_hooks")
    hook = _ntff_profile_via_ctypes("/opt/axon/libaxon_pjrt.so")
    mod.get_axon_ntff_profile_hook = lambda: hook
    mod.set_axon_ntff_profile_hook = lambda h: None
    sys.modules["antenv.axon_hooks"] = mod
    import antenv

    antenv.axon_hooks = mod
    from concourse import bass_utils as _bu

    _bu.upload_artifacts = lambda tmpdir: tmpdir  # keep artifacts local


def kernel_profiled(**inputs):
    """Returns (output, exec_time_ns) using the NTFF profile of core 0."""
    _install_profile_hook()
    out, res = _run(inputs, trace=True)
    return out, res.exec_time_ns


# revision 14
# speedup vs baseline: 1.0766x; 1.0766x over previous
"""Causal self-attention (B=2, T=2048, C=1024, H=16) on 8 TRN2 NeuronCores.

Megatron-style tensor parallelism over heads: each core computes 2 of the 16
heads (Wq/Wk/Wv column-sharded, Wo row-sharded) and produces a partial output
projection; the partials are summed on the host (the all-reduce).

Per-core device dataflow (everything kept transposed so the PE contraction dim
is always the partition dim, no on-device transposes of x needed):
  xT [C, B*T] (host-pretransposed, replicated)
  QT/KT/VT = W_locT.T @ xT  (fp32r matmuls, K-tiled over C)
  V tiles   = PE-transpose of VT, with a ones-column appended per head
  S^T       = K_loc @ Q_loc^T per (batch, head, 128-j-tile, 512-i-block),
              2 heads packed in PE row groups (contraction = D = 64)
  P^T       = exp(S^T / 8) on ACT (no max-subtraction needed: |S|<~2),
              causal mask applied to diagonal block-tiles via mask multiply
  O^T|s     = [V|1].T @ P^T accumulated over j (ones row gives softmax sums)
  ylocT     = O^T * (1/s)  (reciprocal + PE broadcast of 1/s over partitions)
  yT_part   = Wo_locT.T @ ylocT   -> DRAM [C, B*T]
Host: y = (sum_cores yT_part).T + bo, reshape to [B, T, C].
"""

import sys

if "/opt/trn_rl_repo" not in sys.path:
    sys.path.insert(0, "/opt/trn_rl_repo")

import numpy as np

import concourse.bass as bass
import concourse.tile as tile
from concourse import bacc
from concourse import mybir
from concourse.bass_utils import run_bass_kernel_spmd

F32 = mybir.dt.float32
F32R = mybir.dt.float32r
AF = mybir.ActivationFunctionType
ALU = mybir.AluOpType

B, T, C, H = 2, 2048, 1024, 16
D = C // H          # 64
NCORES = 8
HL = H // NCORES    # 2 local heads
CL = C // NCORES    # 128 local channels
BT = B * T          # 4096
TB = 512            # t-block (matmul moving width, fp32 max)
NTB = BT // TB      # 8
NKT = C // 128      # 8 contraction tiles for projections
IB = T // TB        # 4 i-blocks per batch
NJT = T // 128      # 16 j-tiles per batch
VW = 130            # V tile width: 2 heads x (64 V cols + 1 ones col)


def build_nc() -> bass.Bass:
    nc = bacc.Bacc()

    xT_d = nc.declare_dram_parameter("xT", [C, BT], F32R, isOutput=False)
    wqT_d = nc.declare_dram_parameter("wqT", [128, C], F32R, isOutput=False)
    wkT_d = nc.declare_dram_parameter("wkT", [128, C], F32R, isOutput=False)
    wvT_d = nc.declare_dram_parameter("wvT", [128, C], F32R, isOutput=False)
    woT_d = nc.declare_dram_parameter("woT", [CL, C], F32R, isOutput=False)
    bq_d = nc.declare_dram_parameter("bq", [CL, 1], F32, isOutput=False)
    bk_d = nc.declare_dram_parameter("bk", [CL, 1], F32, isOutput=False)
    bv_d = nc.declare_dram_parameter("bv", [CL, 1], F32, isOutput=False)
    mask_d = nc.declare_dram_parameter("masks", [128, 4 * 1024], F32R, isOutput=False)
    id_d = nc.declare_dram_parameter("ident", [128, 128], F32R, isOutput=False)
    yT_d = nc.declare_dram_parameter("yT", [C, BT], F32, isOutput=True)

    with tile.TileContext(nc) as tc:
        with (
            tc.tile_pool(name="const", bufs=1) as const,
            tc.tile_pool(name="work", bufs=2) as work,
            tc.tile_pool(name="psum", bufs=2, space="PSUM") as psum,
        ):
            # ---------------- constants / persistent state ----------------
            wq_sb = const.tile([128, C], F32R)
            wk_sb = const.tile([128, C], F32R)
            wv_sb = const.tile([128, C], F32R)
            nc.sync.dma_start(wq_sb[:, :], wqT_d[:, :])
            nc.sync.dma_start(wk_sb[:, :], wkT_d[:, :])
            nc.sync.dma_start(wv_sb[:, :], wvT_d[:, :])
            wo_sb = const.tile([128, C], F32R)
            nc.sync.dma_start(wo_sb[:, :], woT_d[:, :])
            mask_sb = const.tile([128, 4 * 1024], F32R)
            nc.sync.dma_start(mask_sb[:, :], mask_d[:, :])
            id_sb = const.tile([128, 128], F32R)
            nc.sync.dma_start(id_sb[:, :], id_d[:, :])
            bq_sb = const.tile([128, 1], F32)
            nc.sync.dma_start(bq_sb[:, :], bq_d[:, :])
            bk_sb = const.tile([128, 1], F32)
            nc.sync.dma_start(bk_sb[:, :], bk_d[:, :])
            bv_sb = const.tile([128, 1], F32)
            nc.sync.dma_start(bv_sb[:, :], bv_d[:, :])
            ones_sb = const.tile([1, 64], F32)
            nc.vector.memset(ones_sb[:, :], 1.0)

            QT = const.tile([128, BT], F32R)
            KT = const.tile([128, BT], F32R)
            ylocT = const.tile([128, BT], F32R)
            V = const.tile([128, (BT // 128) * VW], F32R)
            # 1.0f bit pattern; ones columns survive the V copies below
            nc.gpsimd.memset(V[:, :].bitcast(mybir.dt.uint32), 0x3F800000)

            # ---------------- phase 1: Q/K/V projections -------------------
            for tb in range(NTB):
                tcols = slice(tb * TB, (tb + 1) * TB)
                xt = work.tile([128, NKT * TB], F32R, tag="xt", bufs=3,
                               name=f"xt_{tb}")
                xv = xT_d[:, tcols].rearrange("(ct p) t -> p ct t", p=128)
                xo = xt[:, :].rearrange("p (ct t) -> p ct t", ct=NKT)
                nc.sync.dma_start(xo[:, 0:NKT // 2, :], xv[:, 0:NKT // 2, :])
                nc.sync.dma_start(xo[:, NKT // 2:, :], xv[:, NKT // 2:, :])
                for which, w_sb, b_sb in (
                    ("q", wq_sb, bq_sb), ("k", wk_sb, bk_sb), ("v", wv_sb, bv_sb)
                ):
                    ps = psum.tile([128, TB], F32, tag="mm", name=f"ps_{which}_{tb}")
                    for ct in range(NKT):
                        nc.tensor.matmul(
                            ps[:, :],
                            w_sb[:, ct * 128:(ct + 1) * 128],
                            xt[:, ct * TB:(ct + 1) * TB],
                            start=(ct == 0), stop=(ct == NKT - 1),
                        )
                    if which == "q":
                        nc.vector.tensor_scalar_add(QT[:, tcols], ps[:, :], b_sb[:, :])
                    elif which == "k":
                        nc.vector.tensor_scalar_add(KT[:, tcols], ps[:, :], b_sb[:, :])
                    else:
                        vt_sb = work.tile([128, TB], F32R, tag="vtsb",
                                          name=f"vt_{tb}")
                        nc.vector.tensor_scalar_add(vt_sb[:, :], ps[:, :], b_sb[:, :])
                        for q in range(4):
                            jg = tb * 4 + q
                            tp = psum.tile([128, 128], F32R, tag="mm",
                                           name=f"tp_{jg}")
                            nc.tensor.transpose(
                                tp[:, :],
                                vt_sb[:, q * 128:(q + 1) * 128],
                                id_sb[:, :],
                            )
                            off = jg * VW
                            nc.vector.tensor_copy(V[:, off:off + 64], tp[:, 0:64])
                            nc.vector.tensor_copy(V[:, off + 65:off + 129],
                                                  tp[:, 64:128])

            # ------- phase 2+3: attention + output projection per i-block ----
            for b in range(B):
                for ib in range(IB):
                    i0 = b * T + ib * TB
                    icols = slice(i0, i0 + TB)
                    njt = 4 * (ib + 1)
                    ots = [
                        psum.tile([65, TB], F32, tag="ot", name=f"ot_{b}_{ib}_{h}")
                        for h in range(HL)
                    ]
                    for jt in range(njt):
                        jg = b * NJT + jt
                        st = psum.tile([128, 2 * TB], F32, tag="st",
                                       name=f"st_{b}_{ib}_{jt}")
                        for h in range(HL):
                            hs = slice(h * D, (h + 1) * D)
                            nc.tensor.matmul(
                                st[:, h * TB:(h + 1) * TB],
                                KT[hs, jg * 128:(jg + 1) * 128],
                                QT[hs, icols],
                                start=True, stop=True,
                            )
                        pt = work.tile([128, 2 * TB], F32R, tag="pt", bufs=4,
                                       name=f"pt_{b}_{ib}_{jt}")
                        nc.scalar.activation(pt[:, :], st[:, :], AF.Exp,
                                             scale=0.125)
                        if jt >= njt - 4:
                            q = jt - (njt - 4)
                            nc.vector.tensor_mul(
                                pt[:, :], pt[:, :],
                                mask_sb[:, q * 1024:(q + 1) * 1024])
                        for h in range(HL):
                            off = jg * VW + h * 65
                            nc.tensor.matmul(
                                ots[h][:, :],
                                V[:, off:off + 65],
                                pt[:, h * TB:(h + 1) * TB],
                                start=(jt == 0), stop=(jt == njt - 1),
                            )
                    for h in range(HL):
                        s_sb = work.tile([1, TB], F32, tag="s", name=f"s_{b}_{ib}_{h}")
                        nc.vector.tensor_copy(s_sb[:, :], ots[h][64:65, :])
                        r_sb = work.tile([1, TB], F32, tag="r", name=f"r_{b}_{ib}_{h}")
                        nc.vector.reciprocal_approx_fast(r_sb[:, :], s_sb[:, :])
                        bc = psum.tile([64, TB], F32, tag="mm",
                                       name=f"bc_{b}_{ib}_{h}")
                        nc.tensor.matmul(bc[:, :], ones_sb[:, :], r_sb[:, :],
                                         start=True, stop=True)
                        bc_sb = work.tile([64, TB], F32, tag="bcsb",
                                          name=f"bcs_{b}_{ib}_{h}")
                        nc.vector.tensor_copy(bc_sb[:, :], bc[:, :])
                        bc = bc_sb
                        if h == 0:
                            nc.vector.tensor_tensor(
                                ylocT[0:64, icols], ots[h][0:64, :], bc[:, :],
                                ALU.mult)
                        else:
                            yn1 = work.tile([64, TB], F32R, tag="yn1",
                                            name=f"yn_{b}_{ib}")
                            nc.vector.tensor_tensor(
                                yn1[:, :], ots[h][0:64, :], bc[:, :], ALU.mult)
                            nc.sync.dma_start(ylocT[64:128, icols], yn1[:, :])
                    # output projection for this i-block's 512 tokens
                    for co in range(8):
                        yp = psum.tile([128, TB], F32, tag="mm",
                                       name=f"yp_{b}_{ib}_{co}")
                        nc.tensor.matmul(
                            yp[:, :],
                            wo_sb[:, co * 128:(co + 1) * 128],
                            ylocT[:, icols],
                            start=True, stop=True,
                        )
                        yo = work.tile([128, TB], F32, tag="yo", bufs=3,
                                       name=f"yo_{b}_{ib}_{co}")
                        nc.vector.tensor_copy(yo[:, :], yp[:, :])
                        nc.sync.dma_start(yT_d[co * 128:(co + 1) * 128, icols],
                                          yo[:, :])
    nc.compile()
    return nc


def _host_inputs(x, Wq, bq, Wk, bk, Wv, bv, Wo):
    """Build the 8 per-core input maps (host-side layout prep + sharding)."""
    xT = np.ascontiguousarray(x.reshape(BT, C).T.astype(np.float32))
    masks = np.zeros((128, 4 * 1024), np.float32)
    jj = np.arange(128, dtype=np.int32)[:, None]
    ii = np.arange(TB, dtype=np.int32)[None, :]
    for q in range(4):
        m = (ii >= 128 * q + jj).astype(np.float32)
        masks[:, q * 1024:q * 1024 + TB] = m
        masks[:, q * 1024 + TB:(q + 1) * 1024] = m
    ident = np.eye(128, dtype=np.float32)

    def wtile(W, rows):
        # device layout: w_sb[p, k*128 + j] = W[rows][j, k*128 + p]
        wT = W[rows, :].T.astype(np.float32)          # [C, CL]
        return np.ascontiguousarray(
            wT.reshape(NKT, 128, CL).transpose(1, 0, 2).reshape(128, NKT * CL))

    in_maps = []
    for core in range(NCORES):
        rows = slice(core * CL, (core + 1) * CL)
        in_maps.append({
            "xT": xT,
            "wqT": wtile(Wq, rows),
            "wkT": wtile(Wk, rows),
            "wvT": wtile(Wv, rows),
            "woT": np.ascontiguousarray(Wo[:, rows].T.astype(np.float32)),
            "bq": np.ascontiguousarray(bq[rows].reshape(CL, 1).astype(np.float32)),
            "bk": np.ascontiguousarray(bk[rows].reshape(CL, 1).astype(np.float32)),
            "bv": np.ascontiguousarray(bv[rows].reshape(CL, 1).astype(np.float32)),
            "masks": masks,
            "ident": ident,
        })
    return in_maps


_NC_CACHE = None


def _get_nc():
    global _NC_CACHE
    if _NC_CACHE is None:
        _NC_CACHE = build_nc()
    return _NC_CACHE


def _run(inputs, trace=False):
    x = np.asarray(inputs["x"], np.float32)
    in_maps = _host_inputs(
        x,
        np.asarray(inputs["Wq"], np.float32), np.asarray(inputs["bq"], np.float32),
        np.asarray(inputs["Wk"], np.float32), np.asarray(inputs["bk"], np.float32),
        np.asarray(inputs["Wv"], np.float32), np.asarray(inputs["bv"], np.float32),
        np.asarray(inputs["Wo"], np.float32),
    )
    res = run_bass_kernel_spmd(_get_nc(), in_maps, list(range(NCORES)), trace=trace)
    yT = np.zeros((C, BT), np.float64)
    for core in range(NCORES):
        yT += res.results[core]["yT"].astype(np.float64)
    y = yT.T.astype(np.float32) + np.asarray(inputs["bo"], np.float32)
    return y.reshape(B, T, C), res


def kernel(**inputs) -> np.ndarray:
    out, _ = _run(inputs, trace=False)
    return out


def _install_profile_hook():
    """Register the axon NTFF profile hook (the agent image ships the ctypes
    shim in trn_agent_boot but lacks the antenv.axon_hooks module)."""
    import types

    if "antenv.axon_hooks" in sys.modules:
        return
    sys.path.insert(0, "/root/.axon_site")
    from trn_agent_boot.trn_boot import _ntff_profile_via_ctypes

    mod = types.ModuleType("antenv.axon_hooks")
    hook = _ntff_profile_via_ctypes("/opt/axon/libaxon_pjrt.so")
    mod.get_axon_ntff_profile_hook = lambda: hook
    mod.set_axon_ntff_profile_hook = lambda h: None
    sys.modules["antenv.axon_hooks"] = mod
    import antenv

    antenv.axon_hooks = mod
    from concourse import bass_utils as _bu

    _bu.upload_artifacts = lambda tmpdir: tmpdir  # keep artifacts local


def kernel_profiled(**inputs):
    """Returns (output, exec_time_ns) using the NTFF profile of core 0."""
    _install_profile_hook()
    out, res = _run(inputs, trace=True)
    return out, res.exec_time_ns


# revision 15
# speedup vs baseline: 1.2597x; 1.1700x over previous
"""Causal self-attention (B=2, T=2048, C=1024, H=16) on 8 TRN2 NeuronCores.

Megatron-style tensor parallelism over heads: each core computes 2 of the 16
heads (Wq/Wk/Wv column-sharded, Wo row-sharded) and produces a partial output
projection; the partials are summed on the host (the all-reduce).

Per-core device dataflow (everything kept transposed so the PE contraction dim
is always the partition dim, no on-device transposes of x needed):
  xT [C, B*T] (host-pretransposed, replicated)
  QT/KT/VT = W_locT.T @ xT  (fp32r matmuls, K-tiled over C)
  V tiles   = PE-transpose of VT, with a ones-column appended per head
  S^T       = K_loc @ Q_loc^T per (batch, head, 128-j-tile, 512-i-block),
              2 heads packed in PE row groups (contraction = D = 64)
  P^T       = exp(S^T / 8) on ACT (no max-subtraction needed: |S|<~2),
              causal mask applied to diagonal block-tiles via mask multiply
  O^T|s     = [V|1].T @ P^T accumulated over j (ones row gives softmax sums)
  ylocT     = O^T * (1/s)  (reciprocal + PE broadcast of 1/s over partitions)
  yT_part   = Wo_locT.T @ ylocT   -> DRAM [C, B*T]
Host: y = (sum_cores yT_part).T + bo, reshape to [B, T, C].
"""

import sys

if "/opt/trn_rl_repo" not in sys.path:
    sys.path.insert(0, "/opt/trn_rl_repo")

import numpy as np

import concourse.bass as bass
import concourse.tile as tile
from concourse import bacc
from concourse import mybir
from concourse.bass_utils import run_bass_kernel_spmd

F32 = mybir.dt.float32
F32R = mybir.dt.float32r
BF16 = mybir.dt.bfloat16
AF = mybir.ActivationFunctionType
ALU = mybir.AluOpType

B, T, C, H = 2, 2048, 1024, 16
D = C // H          # 64
NCORES = 8
HL = H // NCORES    # 2 local heads
CL = C // NCORES    # 128 local channels
BT = B * T          # 4096
TB = 512            # t-block (matmul moving width, fp32 max)
NTB = BT // TB      # 8
NKT = C // 128      # 8 contraction tiles for projections
IB = T // TB        # 4 i-blocks per batch
NJT = T // 128      # 16 j-tiles per batch
VW = 130            # V tile width: 2 heads x (64 V cols + 1 ones col)


def build_nc() -> bass.Bass:
    nc = bacc.Bacc()

    xT_d = nc.declare_dram_parameter("xT", [C, BT], BF16, isOutput=False)
    wqT_d = nc.declare_dram_parameter("wqT", [128, C], BF16, isOutput=False)
    wkT_d = nc.declare_dram_parameter("wkT", [128, C], BF16, isOutput=False)
    wvT_d = nc.declare_dram_parameter("wvT", [128, C], BF16, isOutput=False)
    woT_d = nc.declare_dram_parameter("woT", [CL, C], BF16, isOutput=False)
    bq_d = nc.declare_dram_parameter("bq", [CL, 1], F32, isOutput=False)
    bk_d = nc.declare_dram_parameter("bk", [CL, 1], F32, isOutput=False)
    bv_d = nc.declare_dram_parameter("bv", [CL, 1], F32, isOutput=False)
    mask_d = nc.declare_dram_parameter("masks", [128, 4 * 1024], BF16, isOutput=False)
    id_d = nc.declare_dram_parameter("ident", [128, 128], BF16, isOutput=False)
    yT_d = nc.declare_dram_parameter("yT", [C, BT], F32, isOutput=True)

    with tile.TileContext(nc) as tc:
        with (
            tc.tile_pool(name="const", bufs=1) as const,
            tc.tile_pool(name="work", bufs=2) as work,
            tc.tile_pool(name="psum", bufs=2, space="PSUM") as psum,
        ):
            # ---------------- constants / persistent state ----------------
            wq_sb = const.tile([128, C], BF16)
            wk_sb = const.tile([128, C], BF16)
            wv_sb = const.tile([128, C], BF16)
            nc.sync.dma_start(wq_sb[:, :], wqT_d[:, :])
            nc.sync.dma_start(wk_sb[:, :], wkT_d[:, :])
            nc.sync.dma_start(wv_sb[:, :], wvT_d[:, :])
            wo_sb = const.tile([128, C], BF16)
            nc.sync.dma_start(wo_sb[:, :], woT_d[:, :])
            mask_sb = const.tile([128, 4 * 1024], BF16)
            nc.sync.dma_start(mask_sb[:, :], mask_d[:, :])
            id_sb = const.tile([128, 128], BF16)
            nc.sync.dma_start(id_sb[:, :], id_d[:, :])
            bq_sb = const.tile([128, 1], F32)
            nc.sync.dma_start(bq_sb[:, :], bq_d[:, :])
            bk_sb = const.tile([128, 1], F32)
            nc.sync.dma_start(bk_sb[:, :], bk_d[:, :])
            bv_sb = const.tile([128, 1], F32)
            nc.sync.dma_start(bv_sb[:, :], bv_d[:, :])
            ones_sb = const.tile([1, 64], F32)
            nc.vector.memset(ones_sb[:, :], 1.0)

            QT = const.tile([128, BT], BF16)
            KT = const.tile([128, BT], BF16)
            ylocT = const.tile([128, BT], BF16)
            V = const.tile([128, (BT // 128) * VW], BF16)
            # 1.0f bit pattern; ones columns survive the V copies below
            nc.gpsimd.memset(V[:, :].bitcast(mybir.dt.uint16), 0x3F80)

            # ---------------- phase 1: Q/K/V projections -------------------
            for tb in range(NTB):
                tcols = slice(tb * TB, (tb + 1) * TB)
                xt = work.tile([128, NKT * TB], BF16, tag="xt", bufs=3,
                               name=f"xt_{tb}")
                xv = xT_d[:, tcols].rearrange("(ct p) t -> p ct t", p=128)
                xo = xt[:, :].rearrange("p (ct t) -> p ct t", ct=NKT)
                nc.sync.dma_start(xo[:, 0:NKT // 2, :], xv[:, 0:NKT // 2, :])
                nc.sync.dma_start(xo[:, NKT // 2:, :], xv[:, NKT // 2:, :])
                for which, w_sb, b_sb in (
                    ("q", wq_sb, bq_sb), ("k", wk_sb, bk_sb), ("v", wv_sb, bv_sb)
                ):
                    ps = psum.tile([128, TB], F32, tag="mm", name=f"ps_{which}_{tb}")
                    for ct in range(NKT):
                        nc.tensor.matmul(
                            ps[:, :],
                            w_sb[:, ct * 128:(ct + 1) * 128],
                            xt[:, ct * TB:(ct + 1) * TB],
                            start=(ct == 0), stop=(ct == NKT - 1),
                        )
                    if which == "q":
                        nc.vector.tensor_scalar_add(QT[:, tcols], ps[:, :], b_sb[:, :])
                    elif which == "k":
                        nc.vector.tensor_scalar_add(KT[:, tcols], ps[:, :], b_sb[:, :])
                    else:
                        vt_sb = work.tile([128, TB], BF16, tag="vtsb",
                                          name=f"vt_{tb}")
                        nc.vector.tensor_scalar_add(vt_sb[:, :], ps[:, :], b_sb[:, :])
                        for q in range(4):
                            jg = tb * 4 + q
                            tp = psum.tile([128, 128], BF16, tag="mm",
                                           name=f"tp_{jg}")
                            nc.tensor.transpose(
                                tp[:, :],
                                vt_sb[:, q * 128:(q + 1) * 128],
                                id_sb[:, :],
                            )
                            off = jg * VW
                            nc.vector.tensor_copy(V[:, off:off + 64], tp[:, 0:64])
                            nc.vector.tensor_copy(V[:, off + 65:off + 129],
                                                  tp[:, 64:128])

            # ------- phase 2+3: attention + output projection per i-block ----
            for b in range(B):
                for ib in range(IB):
                    i0 = b * T + ib * TB
                    icols = slice(i0, i0 + TB)
                    njt = 4 * (ib + 1)
                    ots = [
                        psum.tile([65, TB], F32, tag="ot", name=f"ot_{b}_{ib}_{h}")
                        for h in range(HL)
                    ]
                    for jt in range(njt):
                        jg = b * NJT + jt
                        st = psum.tile([128, 2 * TB], F32, tag="st",
                                       name=f"st_{b}_{ib}_{jt}")
                        for h in range(HL):
                            hs = slice(h * D, (h + 1) * D)
                            nc.tensor.matmul(
                                st[:, h * TB:(h + 1) * TB],
                                KT[hs, jg * 128:(jg + 1) * 128],
                                QT[hs, icols],
                                start=True, stop=True,
                            )
                        pt = work.tile([128, 2 * TB], BF16, tag="pt", bufs=4,
                                       name=f"pt_{b}_{ib}_{jt}")
                        nc.scalar.activation(pt[:, :], st[:, :], AF.Exp,
                                             scale=0.125)
                        if jt >= njt - 4:
                            q = jt - (njt - 4)
                            nc.vector.tensor_mul(
                                pt[:, :], pt[:, :],
                                mask_sb[:, q * 1024:(q + 1) * 1024])
                        for h in range(HL):
                            off = jg * VW + h * 65
                            nc.tensor.matmul(
                                ots[h][:, :],
                                V[:, off:off + 65],
                                pt[:, h * TB:(h + 1) * TB],
                                start=(jt == 0), stop=(jt == njt - 1),
                            )
                    for h in range(HL):
                        s_sb = work.tile([1, TB], F32, tag="s", name=f"s_{b}_{ib}_{h}")
                        nc.vector.tensor_copy(s_sb[:, :], ots[h][64:65, :])
                        r_sb = work.tile([1, TB], F32, tag="r", name=f"r_{b}_{ib}_{h}")
                        nc.vector.reciprocal_approx_fast(r_sb[:, :], s_sb[:, :])
                        bc = psum.tile([64, TB], F32, tag="mm",
                                       name=f"bc_{b}_{ib}_{h}")
                        nc.tensor.matmul(bc[:, :], ones_sb[:, :], r_sb[:, :],
                                         start=True, stop=True)
                        bc_sb = work.tile([64, TB], F32, tag="bcsb",
                                          name=f"bcs_{b}_{ib}_{h}")
                        nc.vector.tensor_copy(bc_sb[:, :], bc[:, :])
                        bc = bc_sb
                        if h == 0:
                            nc.vector.tensor_tensor(
                                ylocT[0:64, icols], ots[h][0:64, :], bc[:, :],
                                ALU.mult)
                        else:
                            yn1 = work.tile([64, TB], BF16, tag="yn1",
                                            name=f"yn_{b}_{ib}")
                            nc.vector.tensor_tensor(
                                yn1[:, :], ots[h][0:64, :], bc[:, :], ALU.mult)
                            nc.sync.dma_start(ylocT[64:128, icols], yn1[:, :])
                    # output projection for this i-block's 512 tokens
                    for co in range(8):
                        yp = psum.tile([128, TB], F32, tag="mm",
                                       name=f"yp_{b}_{ib}_{co}")
                        nc.tensor.matmul(
                            yp[:, :],
                            wo_sb[:, co * 128:(co + 1) * 128],
                            ylocT[:, icols],
                            start=True, stop=True,
                        )
                        yo = work.tile([128, TB], F32, tag="yo", bufs=3,
                                       name=f"yo_{b}_{ib}_{co}")
                        nc.vector.tensor_copy(yo[:, :], yp[:, :])
                        nc.sync.dma_start(yT_d[co * 128:(co + 1) * 128, icols],
                                          yo[:, :])
    nc.compile()
    return nc


def _host_inputs(x, Wq, bq, Wk, bk, Wv, bv, Wo):
    """Build the 8 per-core input maps (host-side layout prep + sharding)."""
    import ml_dtypes
    bf16 = ml_dtypes.bfloat16
    xT = np.ascontiguousarray(x.reshape(BT, C).T.astype(bf16))
    masks = np.zeros((128, 4 * 1024), "float32")
    jj = np.arange(128, dtype=np.int32)[:, None]
    ii = np.arange(TB, dtype=np.int32)[None, :]
    for q in range(4):
        m = (ii >= 128 * q + jj).astype(np.float32)
        masks[:, q * 1024:q * 1024 + TB] = m
        masks[:, q * 1024 + TB:(q + 1) * 1024] = m
    masks = masks.astype(bf16)
    ident = np.eye(128, dtype=bf16)

    def wtile(W, rows):
        # device layout: w_sb[p, k*128 + j] = W[rows][j, k*128 + p]
        wT = W[rows, :].T.astype(bf16)                # [C, CL]
        return np.ascontiguousarray(
            wT.reshape(NKT, 128, CL).transpose(1, 0, 2).reshape(128, NKT * CL))

    in_maps = []
    for core in range(NCORES):
        rows = slice(core * CL, (core + 1) * CL)
        in_maps.append({
            "xT": xT,
            "wqT": wtile(Wq, rows),
            "wkT": wtile(Wk, rows),
            "wvT": wtile(Wv, rows),
            "woT": np.ascontiguousarray(Wo[:, rows].T.astype(bf16)),
            "bq": np.ascontiguousarray(bq[rows].reshape(CL, 1).astype(np.float32)),
            "bk": np.ascontiguousarray(bk[rows].reshape(CL, 1).astype(np.float32)),
            "bv": np.ascontiguousarray(bv[rows].reshape(CL, 1).astype(np.float32)),
            "masks": masks,
            "ident": ident,
        })
    return in_maps


_NC_CACHE = None


def _get_nc():
    global _NC_CACHE
    if _NC_CACHE is None:
        _NC_CACHE = build_nc()
    return _NC_CACHE


def _run(inputs, trace=False):
    x = np.asarray(inputs["x"], np.float32)
    in_maps = _host_inputs(
        x,
        np.asarray(inputs["Wq"], np.float32), np.asarray(inputs["bq"], np.float32),
        np.asarray(inputs["Wk"], np.float32), np.asarray(inputs["bk"], np.float32),
        np.asarray(inputs["Wv"], np.float32), np.asarray(inputs["bv"], np.float32),
        np.asarray(inputs["Wo"], np.float32),
    )
    res = run_bass_kernel_spmd(_get_nc(), in_maps, list(range(NCORES)), trace=trace)
    yT = np.zeros((C, BT), np.float64)
    for core in range(NCORES):
        yT += res.results[core]["yT"].astype(np.float64)
    y = yT.T.astype(np.float32) + np.asarray(inputs["bo"], np.float32)
    return y.reshape(B, T, C), res


def kernel(**inputs) -> np.ndarray:
    out, _ = _run(inputs, trace=False)
    return out


def _install_profile_hook():
    """Register the axon NTFF profile hook (the agent image ships the ctypes
    shim in trn_agent_boot but lacks the antenv.axon_hooks module)."""
    import types

    if "antenv.axon_hooks" in sys.modules:
        return
    sys.path.insert(0, "/root/.axon_site")
    from trn_agent_boot.trn_boot import _ntff_profile_via_ctypes

    mod = types.ModuleType("antenv.axon_hooks")
    hook = _ntff_profile_via_ctypes("/opt/axon/libaxon_pjrt.so")
    mod.get_axon_ntff_profile_hook = lambda: hook
    mod.set_axon_ntff_profile_hook = lambda h: None
    sys.modules["antenv.axon_hooks"] = mod
    import antenv

    antenv.axon_hooks = mod
    from concourse import bass_utils as _bu

    _bu.upload_artifacts = lambda tmpdir: tmpdir  # keep artifacts local


def kernel_profiled(**inputs):
    """Returns (output, exec_time_ns) using the NTFF profile of core 0."""
    _install_profile_hook()
    out, res = _run(inputs, trace=True)
    return out, res.exec_time_ns
